# revision 1
# baseline (speedup 1.0000x reference)
"""Trainium2 Bass kernel for the quirky MultiHeadAttention module.

Reference computation (S = D = 4096, 16 "heads" that are chunks of 256 ROWS):
    q = x @ Wq.T + bq ; k = x @ Wk.T + bk ; v = x @ Wv.T + bv
    per head h (rows h*256..h*256+255):
        scores = split(v)_h @ split(k)_h.T / 64 ; attn = softmax(scores, -1)
        out_h  = attn @ split(q)_h
    result = concat(out_h) @ Wo.T + bo

Sharding: pure data-parallel over token rows. Each of the 8 cores owns 512
rows = exactly 2 complete "heads"; every stage (projections, attention,
output projection) is row-local given full weights, so no collectives.

Matmul dtypes (all accumulate fp32 in PSUM): the PE streams one moving
column per cycle regardless of fp16/bf16, so every N=512 matmul costs
~213ns = full 78.6 TF/s array rate; fp32/fp32r instead pay a long
serialized LDWEIGHTS (~1.8x slower measured). The softmax-amplified paths
(k/v projections, attention) use fp16 (10-bit mantissa); the q projection
and the output projection, whose rounding error passes through linearly,
use bf16, which hides LDWEIGHTS completely. Measured end-to-end error vs
the fp32 reference: 3.9e-3 of absmax; HW time ~0.994 ms (PE ~94% busy,
~90% of the streaming roofline).

Per-core dataflow:
  phase B/C: kT = (x@Wk.T+bk).T and vT likewise -> resident SBUF
             [d_feat 128, kb, tok 512] fp16 (bias via Identity-activation).
  phase A:   q natural [tok, feat] -> resident SBUF fp16; bias folded into
             the PSUM->SBUF copy as a DVE add of a host-replicated row.
  attention: S.T[j,i] = sum_d k[j,d] v[i,d] accumulated in PSUM,
             E.T = exp(S.T/64) fp16 (no max-subtraction: |logits| <~ 8),
             Z[i] = ones-column matmul over E.T partitions, zinv = 1/Z
             (fp32r), broadcast zinv via fp32r ones-row matmul, normalize
             E.T in place, O.T[d,i] = sum_j q[j,d]*E.T_norm[j,i] -> SBUF.
  final:     out rows = O.T.T @ Wo.T + bo, fp32 result to DRAM.
"""

import numpy as np

import concourse.bass as bass
import concourse.bacc as bacc
import concourse.mybir as mybir
import concourse.tile as tile
from concourse.bass_utils import run_bass_kernel_spmd

F32 = mybir.dt.float32
F32R = mybir.dt.float32r
F16 = mybir.dt.float16
BF16 = mybir.dt.bfloat16
AF = mybir.ActivationFunctionType

D = 4096          # d_model == seq
NCORE = 8
SH = D // NCORE   # 512 token rows per core
KB = D // 128     # 32 contraction blocks of 128
NO = D // 512     # 8 output-feature chunks of 512
SM = SH // 128    # 4 token blocks of 128 per core
SCALE = 1.0 / 64.0  # 1/sqrt(4096)


def _build():
    nc = bacc.Bacc(
        "TRN2",
        target_bir_lowering=False,
        debug=False,
        enable_asserts=False,
        num_devices=NCORE,
    )

    xTp = nc.declare_dram_parameter("xTp", [128, KB, SH], F16, isOutput=False)
    xTpb = nc.declare_dram_parameter("xTpb", [128, KB, SH], BF16, isOutput=False)
    wqp = nc.declare_dram_parameter("wqp", [NO, KB, 128, 512], BF16, isOutput=False)
    wkp = nc.declare_dram_parameter("wkp", [KB, 128, KB, 128], F16, isOutput=False)
    wvp = nc.declare_dram_parameter("wvp", [KB, 128, KB, 128], F16, isOutput=False)
    wop = nc.declare_dram_parameter("wop", [NO, KB, 128, 512], BF16, isOutput=False)
    bqb = nc.declare_dram_parameter("bqb", [128, D], F16, isOutput=False)
    bk_p = nc.declare_dram_parameter("bk_p", [128, KB], F32, isOutput=False)
    bv_p = nc.declare_dram_parameter("bv_p", [128, KB], F32, isOutput=False)
    bob = nc.declare_dram_parameter("bob", [128, D], F16, isOutput=False)
    ones16_c = nc.declare_dram_parameter("ones16_c", [128, 1], F16, isOutput=False)
    ones32_r = nc.declare_dram_parameter("ones32_r", [1, 128], F32, isOutput=False)
    out = nc.declare_dram_parameter("out", [SH, D], F32, isOutput=True)

    with tile.TileContext(nc) as tc:
        with (
            nc.allow_low_precision(reason="fp16 matmul operands, fp32 accumulate"),
            tc.tile_pool(name="const", bufs=1) as cpool,
        ):
            ones_col = cpool.tile([128, 1], F16, name="ones_col")
            nc.sync.dma_start(ones_col[:], ones16_c[:])
            ones_row32 = cpool.tile([1, 128], F32R, name="ones_row32")
            nc.sync.dma_start(ones_row32[:], ones32_r[:].bitcast(F32R))
            zero_col = cpool.tile([128, 1], F32, name="zero_col")
            nc.vector.memset(zero_col[:], 0.0)
            bkv = cpool.tile([128, 2 * KB], F32, name="bkv")
            nc.sync.dma_start(bkv[:, 0:KB], bk_p[:])
            nc.sync.dma_start(bkv[:, KB : 2 * KB], bv_p[:])

            with tc.tile_pool(name="otp", bufs=1) as otpool:
              OT = otpool.tile([128, KB, SH], BF16, name="OT")
              with tc.tile_pool(name="kqv", bufs=1) as kqvpool:
                kT = kqvpool.tile([128, KB, SH], F16, name="kT")
                vT = kqvpool.tile([128, KB, SH], F16, name="vT")
                qn = kqvpool.tile([128, SM, D], F16, name="qn")

                with tc.tile_pool(name="etp", bufs=4) as etpool:
                  _ps_ctx = (
                      tc.tile_pool(name="psbc", bufs=4, space="PSUM"),
                      tc.tile_pool(name="psS", bufs=2, space="PSUM"),
                      tc.tile_pool(name="psZ", bufs=1, space="PSUM"),
                      tc.tile_pool(name="psB", bufs=1, space="PSUM"),
                  )
                  psbc_pool = _ps_ctx[0].__enter__()
                  psS_pool = _ps_ctx[1].__enter__()
                  psZ_pool = _ps_ctx[2].__enter__()
                  psB_pool = _ps_ctx[3].__enter__()
                  with tc.tile_pool(name="xpool", bufs=1) as xpool:
                    xT = xpool.tile([128, KB, SH], F16, name="xT")

                    # ---------------- phase B/C: kT and vT ----------------
                    with (
                        tc.tile_pool(name="wslab", bufs=3) as wslab_pool,
                    ):
                        pre_slabs = []
                        for m in range(3):
                            s0 = wslab_pool.tile(
                                [128, KB, 128], F16, tag="slab", name=f"slab_0_{m}"
                            )
                            for qtr in range(4):
                                nc.sync.dma_start(
                                    s0[:, qtr * 8 : (qtr + 1) * 8, :],
                                    wkp[m][:, qtr * 8 : (qtr + 1) * 8, :],
                                )
                            pre_slabs.append(s0)
                        for kb in range(KB):
                            nc.sync.dma_start(xT[:, kb, :], xTp[:, kb, :])
                        for which, dstw in enumerate(((wkp, kT), (wvp, vT))):
                            wp, dst = dstw
                            for m in range(KB):
                                if which == 0 and m < 3:
                                    slab = pre_slabs[m]
                                else:
                                    slab = wslab_pool.tile(
                                        [128, KB, 128], F16, tag="slab",
                                        name=f"slab_{which}_{m}",
                                    )
                                    nc.sync.dma_start(slab[:], wp[m][:])
                                ps = psbc_pool.tile(
                                    [128, SH], F32, tag="acc",
                                    name=f"pskv_{which}_{m}",
                                )
                                for kb in range(KB):
                                    nc.tensor.matmul(
                                        ps[:],
                                        slab[:, kb, :],
                                        xT[:, kb, :],
                                        start=(kb == 0),
                                        stop=(kb == KB - 1),
                                    )
                                nc.scalar.activation(
                                    dst[:, m, :], ps[:], AF.Identity,
                                    bias=bkv[:, which * KB + m : which * KB + m + 1],
                                )

                  # ------- attention part 1: S.T -> normalized E.T (fp16) -------
                  # Emitted before phase A so the PE chews on S.T matmuls while
                  # xTb (bf16 activations for phase A) loads into the slot just
                  # vacated by xT. PSUM pools for S.T live in banks 4-7,
                  # disjoint from B/C's accumulators, so S.T starts without
                  # waiting for the last B/C drain.
                  if True:
                    ETs = {}
                    if True:
                        for h in range(2):
                            psS = [
                                psS_pool.tile(
                                    [128, 256], F32, tag="ps", name=f"psS_{h}_{jb}"
                                )
                                for jb in range(2)
                            ]
                            for kb in range(KB):
                                for jb in range(2):
                                    nc.tensor.matmul(
                                        psS[jb][:],
                                        kT[
                                            :, kb,
                                            h * 256 + jb * 128 : h * 256 + (jb + 1) * 128,
                                        ],
                                        vT[:, kb, h * 256 : (h + 1) * 256],
                                        start=(kb == 0),
                                        stop=(kb == KB - 1),
                                    )
                            ET = []
                            for jb in range(2):
                                et = etpool.tile(
                                    [128, 256], F16, tag="et", bufs=4,
                                    name=f"et_{h}_{jb}",
                                )
                                nc.scalar.activation(
                                    et[:], psS[jb][:], AF.Exp,
                                    bias=zero_col[:], scale=SCALE,
                                )
                                ET.append(et)
                            psz = psZ_pool.tile(
                                [1, 256], F32, tag="pz", name=f"psz_{h}"
                            )
                            for jb in range(2):
                                nc.tensor.matmul(
                                    psz[:],
                                    ones_col[:],
                                    ET[jb][:],
                                    start=(jb == 0),
                                    stop=(jb == 1),
                                )
                            zinv = etpool.tile(
                                [1, 256], F32R, tag="zi", bufs=2, name=f"zinv_{h}"
                            )
                            nc.vector.reciprocal(zinv[:], psz[:])
                            pzb = psB_pool.tile(
                                [128, 256], F32, tag="pb", name=f"pzb_{h}"
                            )
                            nc.tensor.matmul(pzb[:], ones_row32[:], zinv[:])
                            for jb in range(2):
                                nc.vector.tensor_mul(ET[jb][:], ET[jb][:], pzb[:])
                            ETs[h] = ET

                    for _c in reversed(_ps_ctx):
                        _c.__exit__(None, None, None)

                    # ------- phase A: q natural (bf16) -------
                    with (
                        tc.tile_pool(name="xbpool", bufs=1) as xbpool,
                        tc.tile_pool(name="wa", bufs=6) as wa_pool,
                        tc.tile_pool(name="ba", bufs=1) as ba_pool,
                        tc.tile_pool(name="psa", bufs=8, space="PSUM") as psa_pool,
                    ):
                        xTb = xbpool.tile([128, KB, SH], BF16, name="xTb")
                        for kb in range(KB):
                            nc.sync.dma_start(xTb[:, kb, :], xTpb[:, kb, :])
                        bq_t = ba_pool.tile([128, D], F16, name="bq_t")
                        nc.sync.dma_start(bq_t[:], bqb[:])
                        for n in range(NO):
                            pss = [
                                psa_pool.tile(
                                    [128, 512], F32, tag="acc", name=f"psq_{n}_{m}"
                                )
                                for m in range(SM)
                            ]
                            for kb in range(KB):
                                wt = wa_pool.tile(
                                    [128, 512], BF16, tag="wa", name=f"waq_{n}_{kb}"
                                )
                                nc.sync.dma_start(wt[:], wqp[n, kb][:])
                                for m in range(SM):
                                    nc.tensor.matmul(
                                        pss[m][:],
                                        xTb[:, kb, m * 128 : (m + 1) * 128],
                                        wt[:],
                                        start=(kb == 0),
                                        stop=(kb == KB - 1),
                                    )
                            for m in range(SM):
                                nc.vector.tensor_add(
                                    qn[:, m, n * 512 : (n + 1) * 512],
                                    pss[m][:],
                                    bq_t[:, n * 512 : (n + 1) * 512],
                                )

                    # ------- attention part 2: O.T -------
                    with tc.tile_pool(name="psO", bufs=4, space="PSUM") as psO_pool:
                        for h in range(2):
                            ET = ETs[h]
                            for db in range(KB):
                                pso = psO_pool.tile(
                                    [128, 256], F32, tag="po", name=f"psO_{h}_{db}"
                                )
                                for jb in range(2):
                                    nc.tensor.matmul(
                                        pso[:],
                                        qn[:, h * 2 + jb, db * 128 : (db + 1) * 128],
                                        ET[jb][:],
                                        start=(jb == 0),
                                        stop=(jb == 1),
                                    )
                                nc.vector.tensor_copy(
                                    OT[:, db, h * 256 : (h + 1) * 256], pso[:]
                                )

              # ---------------- final: out = concat @ Wo.T + bo ----------------
              # kqv pool is closed here; OT remains live.
              with (
                  tc.tile_pool(name="wf", bufs=6) as wf_pool,
                  tc.tile_pool(name="stf", bufs=4) as stf_pool,
                  tc.tile_pool(name="bf", bufs=1) as bf_pool,
                  tc.tile_pool(name="psf", bufs=8, space="PSUM") as psf_pool,
              ):
                  bo_t = bf_pool.tile([128, D], F16, name="bo_t")
                  nc.sync.dma_start(bo_t[:], bob[:])
                  for n in range(NO):
                      pss = [
                          psf_pool.tile(
                              [128, 512], F32, tag="acc", name=f"psf_{n}_{m}"
                          )
                          for m in range(SM)
                      ]
                      for kb in range(KB):
                          wt = wf_pool.tile(
                              [128, 512], BF16, tag="wf", name=f"wf_{n}_{kb}"
                          )
                          nc.sync.dma_start(wt[:], wop[n, kb][:])
                          for m in range(SM):
                              nc.tensor.matmul(
                                  pss[m][:],
                                  OT[:, kb, m * 128 : (m + 1) * 128],
                                  wt[:],
                                  start=(kb == 0),
                                  stop=(kb == KB - 1),
                              )
                      for m in range(SM):
                          st = stf_pool.tile(
                              [128, 512], F32, tag="stf", name=f"stf_{n}_{m}"
                          )
                          nc.vector.tensor_add(
                              st[:], pss[m][:], bo_t[:, n * 512 : (n + 1) * 512]
                          )
                          nc.sync.dma_start(
                              out[m * 128 : (m + 1) * 128, n * 512 : (n + 1) * 512],
                              st[:],
                          )

    nc.compile()
    return nc


_NC_CACHE = None


def _pack_inputs(x, Wq, bq, Wk, bk, Wv, bv, Wo, bo):
    import ml_dtypes

    f32 = lambda a: np.ascontiguousarray(np.asarray(a, dtype=np.float32))
    x, Wq, bq, Wk, bk, Wv, bv, Wo, bo = map(
        f32, (x, Wq, bq, Wk, bk, Wv, bv, Wo, bo)
    )
    h = np.float16
    b16 = ml_dtypes.bfloat16
    WqT = np.ascontiguousarray(Wq.T)
    WoT = np.ascontiguousarray(Wo.T)
    shared = {
        "wqp": np.ascontiguousarray(
            WqT.reshape(KB, 128, NO, 512).transpose(2, 0, 1, 3)
        ).astype(b16),
        "wkp": np.ascontiguousarray(
            Wk.reshape(KB, 128, KB, 128).transpose(0, 3, 2, 1)
        ).astype(h),
        "wvp": np.ascontiguousarray(
            Wv.reshape(KB, 128, KB, 128).transpose(0, 3, 2, 1)
        ).astype(h),
        "wop": np.ascontiguousarray(
            WoT.reshape(KB, 128, NO, 512).transpose(2, 0, 1, 3)
        ).astype(b16),
        "bqb": np.ascontiguousarray(
            np.broadcast_to(bq.reshape(1, D), (128, D))
        ).astype(h),
        "bk_p": np.ascontiguousarray(bk.reshape(KB, 128).T),
        "bv_p": np.ascontiguousarray(bv.reshape(KB, 128).T),
        "bob": np.ascontiguousarray(
            np.broadcast_to(bo.reshape(1, D), (128, D))
        ).astype(h),
        "ones16_c": np.ones((128, 1), h),
        "ones32_r": np.ones((1, 128), np.float32),
    }
    in_maps = []
    for c in range(NCORE):
        xs = x[c * SH : (c + 1) * SH]
        xTp_f = np.ascontiguousarray(
            xs.T.reshape(KB, 128, SH).transpose(1, 0, 2)
        )
        in_maps.append(
            {"xTp": xTp_f.astype(h), "xTpb": xTp_f.astype(b16), **shared}
        )
    return in_maps


def run(inputs: dict, trace: bool = False, tmpdir=None):
    """Build (cached), run on 8 cores, return (full_output, BassKernelResults)."""
    global _NC_CACHE
    in_maps = _pack_inputs(**inputs)
    if _NC_CACHE is None:
        _NC_CACHE = _build()
    res = run_bass_kernel_spmd(
        _NC_CACHE, in_maps, list(range(NCORE)), trace=trace, tmpdir=tmpdir
    )
    full = np.concatenate(
        [res.results[c]["out"] for c in range(NCORE)], axis=0
    )
    return full, res


def kernel(x, Wq, bq, Wk, bk, Wv, bv, Wo, bo):
    full, _ = run(
        dict(x=x, Wq=Wq, bq=bq, Wk=Wk, bk=bk, Wv=Wv, bv=bv, Wo=Wo, bo=bo)
    )
    return full

